# revision 48
# baseline (speedup 1.0000x reference)
"""Trainium2 Bass kernel for nn_Evolution_4664334483942 (moe_routing).

Model: per-token relation-specific linear (MoE dispatch) feeding a packed
variable-length-sequence LSTM.

Strategy (data-parallel over sequences, 8 cores, no collectives):
  - Global batch b (0..1023) assigned to core b % 8.  Every core then holds
    128 sequences with lengths 128,127,...,1 (identical structure on every
    core), 8256 tokens each.
  - Host folds W_ih @ W_rel[r].T into per-relation fused weights so the MoE
    projection and the LSTM input projection collapse into ONE GEMM:
        gx[n] = x[n] @ Wfuse[rel_n].T + (W_ih b_rel[rel_n] + b_ih + b_hh)
  - Phase 1 (device): dense bf16 GEMM over rel-sorted 128-token tiles,
    writing gx (bf16) to DRAM.  The per-(chunk,rel) bias is injected with a
    rank-1 ones x bias matmul (start=True) so no vector-engine work is
    needed; PSUM->SBUF copies go on the scalar + vector engines.
  - Tokens are split into TIME CHUNKS; chunk 0 is computed before the LSTM
    and later chunks are interleaved into the LSTM's tensor-engine gaps
    (1 tile/step + forced drain at chunk boundaries).  This keeps the PE
    busy through the per-step scalar/vector tail — which also keeps the
    HAM clock-gate warm (2.4 GHz) instead of re-throttling every step.
  - Phase 2 (device): 128 sequential LSTM steps.  Each step gathers its
    gx rows via indirect DMA (per-core index table = data, so the SPMD
    instruction stream stays core-independent), feeds them into the gates
    PSUM via an identity matmul, accumulates h @ W_hh.T on top (bf16
    operands), applies sigmoid/tanh on ScalarE, c/h updates on VectorE
    (fp32 state), PE-transposes bf16 h for the next step, and streams h
    out to DRAM in bf16.
"""

import numpy as np
import ml_dtypes

import concourse.bass as bass
import concourse.mybir as mybir
import concourse.tile as tile
from concourse import bass_utils
from concourse.masks import make_identity
from bass_rust import add_dep_helper
from concourse.vector_clock import ScopedClock

F32 = mybir.dt.float32
BF16 = mybir.dt.bfloat16
I32 = mybir.dt.int32
AF = mybir.ActivationFunctionType
NPBF16 = ml_dtypes.bfloat16

NCORES = 8

# Problem constants (hardcoded; kernel.py must be self-contained).
D = 512          # hidden dim
R = 8            # relations
T = 128          # max sequence length / LSTM steps
B = 1024         # global sequences
KD = D // 128    # contraction k-tiles
G = 4 * D        # gate width (2048)
NJB = G // 512   # psum banks for gates

# phase-1 time-chunk boundaries.  Chunk 0 runs before the LSTM; later
# chunks interleave into the LSTM's tensor-engine gaps (step t only ever
# gathers rows of its own chunk).  Tile counts per (chunk, rel) are
# computed exactly from the data at runtime (max over cores).
CHUNK_TS = (0, 32, 64, 96, 128)

# Results of the last device run (test harness reads exec_time_ns from here).
LAST_RESULTS = None


# ---------------------------------------------------------------------------
# Walrus in this toolchain accepts only ONE sync-wait command per instruction;
# Tile's wait assignment can attach several.  Peel the extras onto same-engine
# NOPs placed immediately before the offending instruction.
# ---------------------------------------------------------------------------
def _split_waits_in_list(nc, insts, max_waits=1):
    out = []
    for inst in insts:
        si = inst.sync_info
        if si is not None and si.on_wait is not None and len(si.on_wait) > max_waits:
            waits = list(si.on_wait)
            for w in waits[max_waits:]:
                nop = mybir.InstNoOp(
                    name=nc.get_next_instruction_name(), ins=[], outs=[],
                )
                nop.engine = inst.engine
                nop.sync_info = mybir.SyncInfo(on_wait=[w], on_update=[])
                out.append(nop)
            inst.sync_info = mybir.SyncInfo(
                on_wait=waits[:max_waits], on_update=list(si.on_update or [])
            )
        out.append(inst)
    return out


class PatchedTileContext(tile.TileContext):
    def _lower_ordered_insts(self, ordered):
        for bb_name in list(ordered.keys()):
            ordered[bb_name] = _split_waits_in_list(self.nc, ordered[bb_name])
        super()._lower_ordered_insts(ordered)

    def _drain_and_barrier(self, tick_clock, wait_clock):
        nop_inst = self.nc.sync.nop()
        wait_clock.add_sem_waits(
            nop_inst.ins, ScopedClock({None: tick_clock.global_clock})
        )
        si = nop_inst.ins.sync_info
        if si is not None and si.on_wait and len(si.on_wait) > 1:
            waits = list(si.on_wait)
            nop_inst.ins.sync_info = mybir.SyncInfo(
                on_wait=[waits[0]], on_update=list(si.on_update or [])
            )
            for w in waits[1:]:
                extra = self.nc.sync.nop()
                extra.ins.sync_info = mybir.SyncInfo(on_wait=[w], on_update=[])
        self.nc.sync.drain()
        self.nc.all_engine_barrier()
        assert self.sems is not None
        popped = self.nc._tile_sem_poison_stack.pop()
        assert popped is self._sem_poison
        self.nc.clear_and_free_semaphores(list(self.sems.allocated().values()))
        self.nc.all_engine_barrier()


# ---------------------------------------------------------------------------
# Device program (core-independent instruction stream; per-core variation is
# carried entirely by input data: xt tile contents and the gather index table)
# ---------------------------------------------------------------------------
def build_program(tiles_cr, nsteps=T):
    """tiles_cr: list (per chunk) of per-rel tile counts [R]."""
    nchunks = len(tiles_cr)
    assert nchunks == len(CHUNK_TS) - 1

    # physical tile order: chunk-major, then rel, then tile
    tile_info = []          # (chunk_idx, rel)
    for ci in range(nchunks):
        for r in range(R):
            for _ in range(tiles_cr[ci][r]):
                tile_info.append((ci, r))
    ntiles = len(tile_info)
    nrows = ntiles * 128
    nloc = nsteps * (nsteps + 1) // 2

    chunk_ntiles = [sum(tiles_cr[ci]) for ci in range(nchunks)]
    chunk_rows_end = []
    acc = 0
    for ci in range(nchunks):
        acc += chunk_ntiles[ci] * 128
        chunk_rows_end.append(acc)
    chunk_of_t = {}
    for ci in range(nchunks):
        for t in range(CHUNK_TS[ci], CHUNK_TS[ci + 1]):
            chunk_of_t[t] = ci

    nc = bass.Bass(target_bir_lowering=False, debug=False, trn_type="TRN2")

    xt = nc.dram_tensor("xt", [ntiles, 128, KD, 128], BF16, kind="ExternalInput").ap()
    wf = nc.dram_tensor("wf", [R, 128, KD, G], BF16, kind="ExternalInput").ap()
    wh = nc.dram_tensor("wh", [128, KD, G], BF16, kind="ExternalInput").ap()
    bt = nc.dram_tensor("bt", [R, 1, G], BF16, kind="ExternalInput").ap()
    brep = nc.dram_tensor("brep", [R, 128, G], BF16, kind="ExternalInput").ap()
    gidx = nc.dram_tensor("gidx", [128, nsteps], I32, kind="ExternalInput").ap()
    out = nc.dram_tensor("out", [nloc, D], BF16, kind="ExternalOutput").ap()
    gx = nc.dram_tensor("gx", [nrows, G], BF16).ap()

    loc_bs = [nsteps - t for t in range(nsteps)]
    loc_off = np.concatenate([[0], np.cumsum(loc_bs)]).astype(int)

    with PatchedTileContext(nc) as tc:
        with tc.tile_pool(name="p1_xt", bufs=5) as xt_pool, \
             tc.tile_pool(name="p1_br", bufs=3) as br_pool, \
             tc.tile_pool(name="p1_gx", bufs=4) as gxs_pool, \
             tc.tile_pool(name="p1_ps", bufs=3, space="PSUM") as ps1_pool, \
             tc.tile_pool(name="p2_const", bufs=1) as const_pool, \
             tc.tile_pool(name="p2_gx", bufs=3) as gx_pool, \
             tc.tile_pool(name="p2_act", bufs=2) as act_pool, \
             tc.tile_pool(name="p2_st", bufs=1) as st_pool, \
             tc.tile_pool(name="p2_h", bufs=2) as h_pool, \
             tc.tile_pool(name="p2_ht", bufs=2) as ht_pool, \
             tc.tile_pool(name="p2_ps", bufs=4, space="PSUM") as ps2_pool, \
             tc.tile_pool(name="p2_tr", bufs=1, space="PSUM") as tr_pool:

            # identity (bf16): stationary for gx-inject matmuls + transposes
            ident_r = const_pool.tile([128, 128], BF16)
            make_identity(nc, ident_r[:])

            # the fused per-relation weights stay fully resident in SBUF
            # (16.8 MB) — reloading 2 MB per rel-run saturates HBM on top of
            # the xt/gx streams and stalls the PE at every run boundary.
            # One tile per relation, loaded lazily two rel-runs ahead so the
            # 16.8 MB doesn't all compete with the first tiles' xt/gx traffic.
            wf_res = [
                const_pool.tile([128, KD, G], BF16, name=f"wf_res{r}")
                for r in range(R)
            ]
            wf_loaded = set()

            def ensure_wf(r):
                if r not in wf_loaded:
                    nc.scalar.dma_start(wf_res[r][:], wf[r])
                    wf_loaded.add(r)

            # ---------------- phase-1 tile emitter -----------------------
            # rel-runs (maximal groups of consecutive same-rel tiles) so the
            # next run's 2 MB weight DMA can be prefetched a full run early
            runs = []
            run_of_tile = []
            for i, (ci, r) in enumerate(tile_info):
                if not runs or runs[-1][1] != r:
                    runs.append((i, r))
                run_of_tile.append(len(runs) - 1)
            run_bufs = {}
            p1_writes = [[] for _ in range(nchunks)]
            p1_emitted = [0 for _ in range(nchunks)]

            def load_run_weights(ri):
                if ri >= len(runs) or ri in run_bufs:
                    return
                # sync ring (pure loads) — the scalar ring is busy streaming
                # the 16.8 MB resident wf at startup and would delay this,
                # which in turn would block PSUM eviction of the first tiles
                r = runs[ri][1]
                br_sb = br_pool.tile([128, G], BF16, tag="br_sb")
                nc.sync.dma_start(br_sb[:], brep[r])
                run_bufs[ri] = br_sb

            # phase-1 emission is BANK (quarter-tile, ~0.9us) granular: a full
            # tile is ~3.7us of PE work, which overflows a step's PE-idle tail
            # and displaces the next recurrent burst 1:1 in the PE FIFO.
            p1_tile_state = {}

            def emit_p1_bank(i, jb):
                ci, r = tile_info[i]
                ri = run_of_tile[i]
                if jb == 0:
                    load_run_weights(ri)
                    if i == runs[ri][0]:
                        load_run_weights(ri + 1)   # prefetch next run's bias
                        ensure_wf(r)
                        for rj in range(ri + 1, min(ri + 3, len(runs))):
                            ensure_wf(runs[rj][1])
                        if ri - 2 in run_bufs:
                            del run_bufs[ri - 2]
                    xt_sb = xt_pool.tile([128, KD, 128], BF16, tag="xt_sb")
                    nc.sync.dma_start(xt_sb[:], xt[i])
                    gxs = gxs_pool.tile([128, G], BF16, tag="gxs")
                    p1_tile_state[i] = (xt_sb, gxs)
                br_sb = run_bufs[ri]
                xt_sb, gxs = p1_tile_state[i]
                sl = slice(jb * 512, (jb + 1) * 512)
                ps_b = ps1_pool.tile([128, 512], F32, tag="ps1",
                                     name=f"ps1_{jb}")
                for k in range(KD):
                    nc.tensor.matmul(
                        ps_b[:], xt_sb[:, k, :], wf_res[r][:, k, sl],
                        start=(k == 0), stop=(k == KD - 1),
                    )
                # PSUM eviction with fused bias on the vector engine:
                # scalar-queue copies would delay the next step's gate
                # activations, which are chain-critical
                nc.vector.tensor_add(gxs[:, sl], ps_b[:], br_sb[:, sl])
                if jb == 3:
                    del p1_tile_state[i]
                    # gx store on the gpsimd (SWDGE) queue: keeps the sync
                    # queue a pure-load FIFO so weight prefetches are never
                    # stuck behind a store that waits on compute
                    wi = nc.gpsimd.dma_start(gx[i * 128:(i + 1) * 128, :],
                                             gxs[:])
                    p1_writes[ci].append(wi.ins)
                    p1_emitted[ci] += 1

            def emit_p1_tile(i):
                for jb in range(4):
                    emit_p1_bank(i, jb)

            # chunk 0 runs before the LSTM
            n_c0 = chunk_ntiles[0]
            for i in range(n_c0):
                emit_p1_tile(i)
            pending = [(i, jb) for i in range(n_c0, ntiles) for jb in range(4)]

            # deadline-aware pacing (in bank units): chunk ci must be
            # emitted a few steps before its first gather; spread the rest
            # evenly so the PE stays fed (and HAM stays warm) across as many
            # LSTM steps as possible.
            targets = []
            cum = 0
            for ci in range(1, nchunks):
                cum += 4 * chunk_ntiles[ci]
                targets.append((max(0, CHUNK_TS[ci] - 4), cum))
            targets.append((nsteps - 8, cum))   # spread any slack work
            p1_quota = [0] * nsteps
            emitted_plan = 0
            acc = 0.0
            for t in range(nsteps):
                rate = 0.0
                for d, need in targets:
                    if d >= t and need > emitted_plan + acc:
                        rate = max(rate, (need - emitted_plan - acc) / (d - t + 1))
                acc += rate
                n = int(acc)
                acc -= n
                p1_quota[t] = n
                emitted_plan += n

            # ---------------- phase 2: LSTM ------------------------------
            wh_sb = const_pool.tile([128, KD, G], BF16)
            nc.sync.dma_start(wh_sb[:], wh[:])
            idx_sb = const_pool.tile([128, nsteps], I32)
            nc.sync.dma_start(idx_sb[:], gidx[:])

            c_sb = st_pool.tile([128, D], BF16)
            tmp1 = st_pool.tile([128, D], BF16)
            tmp2 = st_pool.tile([128, D], BF16)

            ht_sb = None
            gxt_tiles = {}
            banks = {}

            def emit_gather(t):
                ci = chunk_of_t[t]
                # make sure the chunk's phase-1 tiles are all emitted
                while p1_emitted[ci] < chunk_ntiles[ci] and pending:
                    emit_p1_bank(*pending.pop(0))
                gxt = gx_pool.tile([128, G], BF16, tag="gxt")
                rows_end = chunk_rows_end[ci]
                gi = nc.gpsimd.indirect_dma_start(
                    out=gxt[:],
                    out_offset=None,
                    in_=gx[0:rows_end, :],
                    in_offset=bass.IndirectOffsetOnAxis(
                        ap=idx_sb[:, t:t + 1], axis=0
                    ),
                )
                # the tracker cannot see through the dynamic row offsets, so
                # order the gather after every write of its chunk explicitly
                for w in p1_writes[ci]:
                    add_dep_helper(gi.ins, w, reason="gather waits gx chunk")
                gxt_tiles[t] = gxt

            def emit_ident(t, jb):
                # first write of bank jb for step t: gates <- gx rows
                psb = ps2_pool.tile([128, 512], F32, tag="ps2")
                nc.tensor.matmul(
                    psb[:], ident_r[:],
                    gxt_tiles[t][:, jb * 512:(jb + 1) * 512],
                    start=True, stop=(t == 0),
                )
                banks[(t, jb)] = psb

            # bank processing order: g first so the c-chain overlaps later banks
            BORD = (2, 0, 1, 3)
            emit_gather(0)
            emit_gather(1)
            for jb in BORD:
                emit_ident(0, jb)
            for t in range(nsteps):
                bs = nsteps - t
                if t + 2 < nsteps:
                    emit_gather(t + 2)
                sif = act_pool.tile([128, 2 * D], BF16, tag="sif")
                tg = act_pool.tile([128, D], BF16, tag="tg")
                so = act_pool.tile([128, D], BF16, tag="so")
                act_of = {
                    2: (tg, 0, AF.Tanh),
                    0: (sif, 0, AF.Sigmoid),
                    1: (sif, D, AF.Sigmoid),
                    3: (so, 0, AF.Sigmoid),
                }
                H = D // 2
                # recurrent accumulation, bank-major so the chain-critical
                # gates (g, i, f) complete and activate as early as possible;
                # k 0,1 use half 0 of the transposed h (ready first), k 2,3
                # use half 1
                def rec(jb, ks):
                    sl = slice(jb * 512, (jb + 1) * 512)
                    psb = banks[(t, jb)]
                    for k in ks:
                        nc.tensor.matmul(
                            psb[:],
                            ht_sb[:, k * 128:(k + 1) * 128],
                            wh_sb[:, k, sl],
                            start=False,
                            stop=(k == KD - 1),
                        )

                def act(jb):
                    tile_, base, fn = act_of[jb]
                    nc.scalar.activation(
                        tile_[:, base:base + D], banks.pop((t, jb))[:], fn)

                def act_h(jb, hh):
                    # half-width activation: shortens the chain to the first
                    # c-update half (the second half trails in parallel)
                    tile_, base, fn = act_of[jb]
                    psb = banks[(t, jb)] if hh == 0 else banks.pop((t, jb))
                    nc.scalar.activation(
                        tile_[:, base + hh * H:base + (hh + 1) * H],
                        psb[:, hh * H:(hh + 1) * H], fn)

                if t > 0:
                    # fully bank-serial g: by the time g's k0,k1 matmuls have
                    # drained, cast1 (half 1 of the transposed h) has landed,
                    # so k2,k3 follow back-to-back and act_g fires ~0.6us
                    # earlier than with the banks' first waves interleaved
                    rec(2, (0, 1))
                    rec(2, (2, 3))
                    act(2)
                    rec(0, (0, 1))
                    rec(0, (2, 3))
                    act(0)
                    rec(1, (0, 1, 2, 3))
                    act_h(1, 0)
                    rec(3, (0, 1, 2, 3))
                    # o half 0 before f half 1 in the ACT FIFO: tanh(c0) needs
                    # o0 (via h0) but f1 only gates the second c-half
                    act_h(3, 0)
                    act_h(1, 1)
                else:
                    for jb in BORD:
                        act(jb)
                # inject next step's gx only after the chain-critical matmuls
                # above, so the idents don't displace them in the PE queue
                if t + 1 < nsteps:
                    for jb in BORD:
                        emit_ident(t + 1, jb)
                # slot A for phase-1 banks: right after the idents there is
                # ~1us of PE-idle before the transposes become ready, so one
                # bank (~0.9us) rides free here
                slot_a = 1 if (p1_quota[t] >= 1 and pending) else 0
                if slot_a:
                    emit_p1_bank(*pending.pop(0))

                # c update in halves (all on VectorE: a concurrent GpSimd
                # elementwise op would contend for the shared SBUF port);
                # half 0 completes right after act_f half 0 so tanh(c) and
                # the h/transpose chain start ~0.5us earlier
                if t == 0:
                    nc.vector.tensor_tensor(
                        c_sb[:], sif[:, 0:D], tg[:], mybir.AluOpType.mult
                    )
                else:
                    nc.vector.tensor_tensor(
                        tmp2[:], sif[:, 0:D], tg[:], mybir.AluOpType.mult
                    )
                    for hh in range(2):
                        sl = slice(hh * H, (hh + 1) * H)
                        slf = slice(D + hh * H, D + (hh + 1) * H)
                        nc.vector.tensor_tensor(
                            tmp1[:, sl], sif[:, slf], c_sb[:, sl],
                            mybir.AluOpType.mult
                        )
                        nc.vector.tensor_add(c_sb[:, sl], tmp1[:, sl],
                                             tmp2[:, sl])
                tc_sb = act_pool.tile([128, D], BF16, tag="tc_sb")
                # tanh(c) / h / transpose / cast in halves so the next step's
                # first recurrent matmuls (k=0,1) start as soon as half 0 of
                # the transposed h is ready
                h_sb = h_pool.tile([128, D], BF16, tag="h_sb")
                if t < nsteps - 1:
                    trp = tr_pool.tile([128, D], BF16, tag="trp")
                    new_ht = ht_pool.tile([128, D], BF16, tag="ht_sb")
                for hh in range(2):
                    sl = slice(hh * H, (hh + 1) * H)
                    nc.scalar.activation(tc_sb[:, sl], c_sb[:, sl], AF.Tanh)
                    if hh == 0 and t > 0:
                        act_h(3, 1)   # o half 1, off the h-half-0 path
                    nc.vector.tensor_tensor(
                        h_sb[:, sl], so[:, sl], tc_sb[:, sl],
                        mybir.AluOpType.mult,
                    )
                    if t < nsteps - 1:
                        for k in (2 * hh, 2 * hh + 1):
                            nc.tensor.transpose(
                                trp[:, k * 128:(k + 1) * 128],
                                h_sb[:, k * 128:(k + 1) * 128],
                                ident_r[:],
                            )
                        nc.vector.tensor_copy(new_ht[:, sl], trp[:, sl])
                if t < nsteps - 1:
                    ht_sb = new_ht
                # stream out this step's hidden states (packed rows); scalar
                # (HWDGE) queue so the sync queue stays a pure-load FIFO
                nc.scalar.dma_start(
                    out[int(loc_off[t]):int(loc_off[t]) + bs, :], h_sb[:bs, :]
                )
                # slot B: remaining phase-1 banks for this step (emitted last
                # so their vector-engine evictions queue behind this step's
                # chain-critical c/h ops, not in front of them)
                for _ in range(p1_quota[t] - slot_a):
                    if pending:
                        emit_p1_bank(*pending.pop(0))
    return nc


# ---------------------------------------------------------------------------
# Host-side data marshaling
# ---------------------------------------------------------------------------
def _expected_layout():
    lengths = T - np.arange(B) // NCORES
    batch_sizes = np.array([(lengths > t).sum() for t in range(T)], dtype=np.int32)
    time_idx = np.concatenate(
        [np.full(bs, t, np.int32) for t, bs in enumerate(batch_sizes)]
    )
    batch_idx = np.concatenate(
        [np.arange(bs, dtype=np.int32) for bs in batch_sizes]
    )
    return batch_sizes, time_idx, batch_idx


def _numpy_reference(embed, W_rel, b_rel, W_ih, W_hh, b_ih, b_hh,
                     nodes, rels, time_idx, batch_idx, batch_sizes):
    """Pure-numpy fallback (only used if the packed layout differs from the
    hardcoded one)."""
    n_steps = int(batch_sizes.shape[0])
    max_bs = int(batch_sizes.max())
    x = embed[nodes]
    y = np.zeros_like(x)
    for r in range(W_rel.shape[0]):
        m = rels == r
        y[m] = x[m] @ W_rel[r].T + b_rel[r]
    d = x.shape[-1]
    xp = np.zeros((n_steps, max_bs, d), x.dtype)
    mask = np.zeros((n_steps, max_bs), bool)
    xp[time_idx, batch_idx] = y
    mask[time_idx, batch_idx] = True
    bias = b_ih + b_hh

    def sig(v):
        return 1.0 / (1.0 + np.exp(-v))

    h = np.zeros((max_bs, d), x.dtype)
    c = np.zeros((max_bs, d), x.dtype)
    hs = np.zeros((n_steps, max_bs, d), x.dtype)
    for t in range(n_steps):
        gates = xp[t] @ W_ih.T + h @ W_hh.T + bias
        i, f, g, o = np.split(gates, 4, axis=-1)
        c_new = sig(f) * c + sig(i) * np.tanh(g)
        h_new = sig(o) * np.tanh(c_new)
        m = mask[t][:, None]
        h = np.where(m, h_new, h)
        c = np.where(m, c_new, c)
        hs[t] = h
    return hs[time_idx, batch_idx]


def _prepare_host(inputs, nsteps=T):
    """Build per-core device input dicts + the output unshard map."""
    embed = np.asarray(inputs["embed"], np.float32)
    W_rel = np.asarray(inputs["W_rel"], np.float32)
    b_rel = np.asarray(inputs["b_rel"], np.float32)
    W_ih = np.asarray(inputs["W_ih"], np.float32)
    W_hh = np.asarray(inputs["W_hh"], np.float32)
    b_ih = np.asarray(inputs["b_ih"], np.float32)
    b_hh = np.asarray(inputs["b_hh"], np.float32)
    nodes = np.asarray(inputs["nodes"])
    rels = np.asarray(inputs["rels"])

    nchunks = len(CHUNK_TS) - 1
    nloc = nsteps * (nsteps + 1) // 2

    # fused weights & biases (float64 for accuracy, cast down)
    Wfuse = (W_ih.astype(np.float64) @ W_rel.astype(np.float64))
    Wfuse = Wfuse.astype(np.float32)            # [R, G, D]
    btot = (W_ih.astype(np.float64) @ b_rel.astype(np.float64).T).T \
        + (b_ih + b_hh).astype(np.float64)      # [R, G]
    btot = btot.astype(np.float32)

    wf_host = np.ascontiguousarray(
        Wfuse.transpose(0, 2, 1).reshape(R, KD, 128, G).transpose(0, 2, 1, 3)
    ).astype(NPBF16)                             # [R, 128(dk), KD, G]
    wh_host = np.ascontiguousarray(
        W_hh.T.reshape(KD, 128, G).transpose(1, 0, 2)
    ).astype(NPBF16)                             # [128(dk), KD, G]
    bt_host = np.ascontiguousarray(btot[:, None, :]).astype(NPBF16)  # [R,1,G]
    brep_host = np.ascontiguousarray(
        np.broadcast_to(btot[:, None, :], (R, 128, G))
    ).astype(NPBF16)                             # [R, 128, G]

    # local token enumeration (identical structure for every core)
    t_arr = np.concatenate(
        [np.full(nsteps - t, t, np.int64) for t in range(nsteps)]
    )
    j_arr = np.concatenate(
        [np.arange(nsteps - t, dtype=np.int64) for t in range(nsteps)]
    )
    gbs = NCORES * (nsteps - np.arange(nsteps, dtype=np.int64))
    goff = np.concatenate([[0], np.cumsum(gbs)])

    chunk_of_t = np.zeros(nsteps, np.int64)
    for ci in range(nchunks):
        chunk_of_t[CHUNK_TS[ci]:CHUNK_TS[ci + 1]] = ci
    ch_loc = chunk_of_t[t_arr]

    # per-core per-(chunk,rel) token counts -> exact shared tile budgets
    rel_by_core = []
    counts = np.zeros((NCORES, nchunks, R), np.int64)
    for core in range(NCORES):
        grow = goff[t_arr] + NCORES * j_arr + core
        rel_loc = rels[grow].astype(np.int64)
        rel_by_core.append((grow, rel_loc))
        np.add.at(counts[core], (ch_loc, rel_loc), 1)
    tiles_cr = [
        [int(np.ceil(counts[:, ci, r].max() / 128)) if counts[:, ci, r].max() > 0
         else 0 for r in range(R)]
        for ci in range(nchunks)
    ]

    # segment bases (must mirror build_program's tile order)
    seg_base = {}
    acc_tiles = 0
    for ci in range(nchunks):
        for r in range(R):
            seg_base[(ci, r)] = acc_tiles * 128
            acc_tiles += tiles_cr[ci][r]
    ntiles = acc_tiles

    in_maps = []
    for core in range(NCORES):
        grow, rel_loc = rel_by_core[core]
        node_loc = nodes[grow]

        order = np.lexsort((j_arr, t_arr, rel_loc, ch_loc))
        key = ch_loc[order] * R + rel_loc[order]
        cnt = np.bincount(key, minlength=nchunks * R)
        q = np.concatenate([np.arange(c) for c in cnt])
        base_sorted = np.array(
            [seg_base[(k // R, k % R)] for k in key], np.int64
        )
        prow_sorted = base_sorted + q
        prow = np.empty(nloc, np.int64)
        prow[order] = prow_sorted

        gidx_host = np.zeros((128, nsteps), np.int32)
        gidx_host[j_arr, t_arr] = prow

        Xp = np.zeros((ntiles * 128, D), np.float32)
        Xp[prow] = embed[node_loc]
        xt_host = np.ascontiguousarray(
            Xp.reshape(ntiles, 128, KD, 128).transpose(0, 3, 2, 1)
        ).astype(NPBF16)                         # [NT, 128(dk), KD, 128(tok)]

        in_maps.append({
            "xt": xt_host,
            "wf": wf_host,
            "wh": wh_host,
            "bt": bt_host,
            "brep": brep_host,
            "gidx": gidx_host,
        })

    unshard = {
        "t_arr": t_arr, "j_arr": j_arr, "goff": goff,
        "nloc": nloc,
    }
    return in_maps, unshard, tiles_cr


def kernel(**inputs):
    global LAST_RESULTS
    import os

    # Verify the packed layout matches the hardcoded structure.
    bs_exp, ti_exp, bi_exp = _expected_layout()
    ok = (
        np.array_equal(np.asarray(inputs["batch_sizes"]), bs_exp)
        and np.array_equal(np.asarray(inputs["time_idx"]), ti_exp)
        and np.array_equal(np.asarray(inputs["batch_idx"]), bi_exp)
        and np.asarray(inputs["embed"]).shape == (50000, D)
    )
    if not ok:
        return _numpy_reference(**{k: np.asarray(v) for k, v in inputs.items()})

    in_maps, unshard, tiles_cr = _prepare_host(inputs)

    nc = build_program(tiles_cr)
    trace = bool(os.environ.get("KERNEL_TRACE"))
    res = bass_utils.run_bass_kernel_spmd(
        nc, in_maps, core_ids=list(range(NCORES)), trace=trace,
    )
    LAST_RESULTS = res

    t_arr = unshard["t_arr"]
    j_arr = unshard["j_arr"]
    goff = unshard["goff"]
    out_full = np.zeros((len(np.asarray(inputs["time_idx"])), D), np.float32)
    for core in range(NCORES):
        grow = goff[t_arr] + NCORES * j_arr + core
        out_full[grow] = np.asarray(res.results[core]["out"], np.float32)
    return out_full


# revision 49
# speedup vs baseline: 1.0027x; 1.0027x over previous
"""Trainium2 Bass kernel for nn_Evolution_4664334483942 (moe_routing).

Model: per-token relation-specific linear (MoE dispatch) feeding a packed
variable-length-sequence LSTM.

Strategy (data-parallel over sequences, 8 cores, no collectives):
  - Global batch b (0..1023) assigned to core b % 8.  Every core then holds
    128 sequences with lengths 128,127,...,1 (identical structure on every
    core), 8256 tokens each.
  - Host folds W_ih @ W_rel[r].T into per-relation fused weights so the MoE
    projection and the LSTM input projection collapse into ONE GEMM:
        gx[n] = x[n] @ Wfuse[rel_n].T + (W_ih b_rel[rel_n] + b_ih + b_hh)
  - Phase 1 (device): dense bf16 GEMM over rel-sorted 128-token tiles,
    writing gx (bf16) to DRAM.  The per-(chunk,rel) bias is injected with a
    rank-1 ones x bias matmul (start=True) so no vector-engine work is
    needed; PSUM->SBUF copies go on the scalar + vector engines.
  - Tokens are split into TIME CHUNKS; chunk 0 is computed before the LSTM
    and later chunks are interleaved into the LSTM's tensor-engine gaps
    (1 tile/step + forced drain at chunk boundaries).  This keeps the PE
    busy through the per-step scalar/vector tail — which also keeps the
    HAM clock-gate warm (2.4 GHz) instead of re-throttling every step.
  - Phase 2 (device): 128 sequential LSTM steps.  Each step gathers its
    gx rows via indirect DMA (per-core index table = data, so the SPMD
    instruction stream stays core-independent), feeds them into the gates
    PSUM via an identity matmul, accumulates h @ W_hh.T on top (bf16
    operands), applies sigmoid/tanh on ScalarE, c/h updates on VectorE
    (fp32 state), PE-transposes bf16 h for the next step, and streams h
    out to DRAM in bf16.
"""

import numpy as np
import ml_dtypes

import concourse.bass as bass
import concourse.mybir as mybir
import concourse.tile as tile
from concourse import bass_utils
from concourse.masks import make_identity
from bass_rust import add_dep_helper
from concourse.vector_clock import ScopedClock

F32 = mybir.dt.float32
BF16 = mybir.dt.bfloat16
I32 = mybir.dt.int32
AF = mybir.ActivationFunctionType
NPBF16 = ml_dtypes.bfloat16

NCORES = 8

# Problem constants (hardcoded; kernel.py must be self-contained).
D = 512          # hidden dim
R = 8            # relations
T = 128          # max sequence length / LSTM steps
B = 1024         # global sequences
KD = D // 128    # contraction k-tiles
G = 4 * D        # gate width (2048)
NJB = G // 512   # psum banks for gates

# phase-1 time-chunk boundaries.  Chunk 0 runs before the LSTM; later
# chunks interleave into the LSTM's tensor-engine gaps (step t only ever
# gathers rows of its own chunk).  Tile counts per (chunk, rel) are
# computed exactly from the data at runtime (max over cores).
CHUNK_TS = (0, 32, 64, 96, 128)

# Results of the last device run (test harness reads exec_time_ns from here).
LAST_RESULTS = None


# ---------------------------------------------------------------------------
# Walrus in this toolchain accepts only ONE sync-wait command per instruction;
# Tile's wait assignment can attach several.  Peel the extras onto same-engine
# NOPs placed immediately before the offending instruction.
# ---------------------------------------------------------------------------
def _split_waits_in_list(nc, insts, max_waits=1):
    out = []
    for inst in insts:
        si = inst.sync_info
        if si is not None and si.on_wait is not None and len(si.on_wait) > max_waits:
            waits = list(si.on_wait)
            for w in waits[max_waits:]:
                nop = mybir.InstNoOp(
                    name=nc.get_next_instruction_name(), ins=[], outs=[],
                )
                nop.engine = inst.engine
                nop.sync_info = mybir.SyncInfo(on_wait=[w], on_update=[])
                out.append(nop)
            inst.sync_info = mybir.SyncInfo(
                on_wait=waits[:max_waits], on_update=list(si.on_update or [])
            )
        out.append(inst)
    return out


class PatchedTileContext(tile.TileContext):
    def _lower_ordered_insts(self, ordered):
        for bb_name in list(ordered.keys()):
            ordered[bb_name] = _split_waits_in_list(self.nc, ordered[bb_name])
        super()._lower_ordered_insts(ordered)

    def _drain_and_barrier(self, tick_clock, wait_clock):
        nop_inst = self.nc.sync.nop()
        wait_clock.add_sem_waits(
            nop_inst.ins, ScopedClock({None: tick_clock.global_clock})
        )
        si = nop_inst.ins.sync_info
        if si is not None and si.on_wait and len(si.on_wait) > 1:
            waits = list(si.on_wait)
            nop_inst.ins.sync_info = mybir.SyncInfo(
                on_wait=[waits[0]], on_update=list(si.on_update or [])
            )
            for w in waits[1:]:
                extra = self.nc.sync.nop()
                extra.ins.sync_info = mybir.SyncInfo(on_wait=[w], on_update=[])
        self.nc.sync.drain()
        self.nc.all_engine_barrier()
        assert self.sems is not None
        popped = self.nc._tile_sem_poison_stack.pop()
        assert popped is self._sem_poison
        self.nc.clear_and_free_semaphores(list(self.sems.allocated().values()))
        self.nc.all_engine_barrier()


# ---------------------------------------------------------------------------
# Device program (core-independent instruction stream; per-core variation is
# carried entirely by input data: xt tile contents and the gather index table)
# ---------------------------------------------------------------------------
def build_program(tiles_cr, nsteps=T):
    """tiles_cr: list (per chunk) of per-rel tile counts [R]."""
    nchunks = len(tiles_cr)
    assert nchunks == len(CHUNK_TS) - 1

    # physical tile order: chunk-major, then rel, then tile
    tile_info = []          # (chunk_idx, rel)
    for ci in range(nchunks):
        for r in range(R):
            for _ in range(tiles_cr[ci][r]):
                tile_info.append((ci, r))
    ntiles = len(tile_info)
    nrows = ntiles * 128
    nloc = nsteps * (nsteps + 1) // 2

    chunk_ntiles = [sum(tiles_cr[ci]) for ci in range(nchunks)]
    chunk_rows_end = []
    acc = 0
    for ci in range(nchunks):
        acc += chunk_ntiles[ci] * 128
        chunk_rows_end.append(acc)
    chunk_of_t = {}
    for ci in range(nchunks):
        for t in range(CHUNK_TS[ci], CHUNK_TS[ci + 1]):
            chunk_of_t[t] = ci

    nc = bass.Bass(target_bir_lowering=False, debug=False, trn_type="TRN2")

    xt = nc.dram_tensor("xt", [ntiles, 128, KD, 128], BF16, kind="ExternalInput").ap()
    wf = nc.dram_tensor("wf", [R, 128, KD, G], BF16, kind="ExternalInput").ap()
    wh = nc.dram_tensor("wh", [128, KD, G], BF16, kind="ExternalInput").ap()
    bt = nc.dram_tensor("bt", [R, 1, G], BF16, kind="ExternalInput").ap()
    brep = nc.dram_tensor("brep", [R, 128, G], BF16, kind="ExternalInput").ap()
    gidx = nc.dram_tensor("gidx", [128, nsteps], I32, kind="ExternalInput").ap()
    out = nc.dram_tensor("out", [nloc, D], BF16, kind="ExternalOutput").ap()
    gx = nc.dram_tensor("gx", [nrows, G], BF16).ap()

    loc_bs = [nsteps - t for t in range(nsteps)]
    loc_off = np.concatenate([[0], np.cumsum(loc_bs)]).astype(int)

    with PatchedTileContext(nc) as tc:
        with tc.tile_pool(name="p1_xt", bufs=5) as xt_pool, \
             tc.tile_pool(name="p1_br", bufs=3) as br_pool, \
             tc.tile_pool(name="p1_gx", bufs=4) as gxs_pool, \
             tc.tile_pool(name="p1_ps", bufs=3, space="PSUM") as ps1_pool, \
             tc.tile_pool(name="p2_const", bufs=1) as const_pool, \
             tc.tile_pool(name="p2_gx", bufs=3) as gx_pool, \
             tc.tile_pool(name="p2_act", bufs=2) as act_pool, \
             tc.tile_pool(name="p2_st", bufs=1) as st_pool, \
             tc.tile_pool(name="p2_h", bufs=2) as h_pool, \
             tc.tile_pool(name="p2_ht", bufs=2) as ht_pool, \
             tc.tile_pool(name="p2_ps", bufs=4, space="PSUM") as ps2_pool, \
             tc.tile_pool(name="p2_tr", bufs=1, space="PSUM") as tr_pool:

            # identity (bf16): stationary for gx-inject matmuls + transposes
            ident_r = const_pool.tile([128, 128], BF16)
            make_identity(nc, ident_r[:])

            # the fused per-relation weights stay fully resident in SBUF
            # (16.8 MB) — reloading 2 MB per rel-run saturates HBM on top of
            # the xt/gx streams and stalls the PE at every run boundary.
            # One tile per relation, loaded lazily two rel-runs ahead so the
            # 16.8 MB doesn't all compete with the first tiles' xt/gx traffic.
            wf_res = [
                const_pool.tile([128, KD, G], BF16, name=f"wf_res{r}")
                for r in range(R)
            ]
            wf_loaded = set()

            def ensure_wf(r):
                if r not in wf_loaded:
                    nc.scalar.dma_start(wf_res[r][:], wf[r])
                    wf_loaded.add(r)

            # ---------------- phase-1 tile emitter -----------------------
            # rel-runs (maximal groups of consecutive same-rel tiles) so the
            # next run's 2 MB weight DMA can be prefetched a full run early
            runs = []
            run_of_tile = []
            for i, (ci, r) in enumerate(tile_info):
                if not runs or runs[-1][1] != r:
                    runs.append((i, r))
                run_of_tile.append(len(runs) - 1)
            run_bufs = {}
            p1_writes = [[] for _ in range(nchunks)]
            p1_emitted = [0 for _ in range(nchunks)]

            def load_run_weights(ri):
                if ri >= len(runs) or ri in run_bufs:
                    return
                # sync ring (pure loads) — the scalar ring is busy streaming
                # the 16.8 MB resident wf at startup and would delay this,
                # which in turn would block PSUM eviction of the first tiles
                r = runs[ri][1]
                br_sb = br_pool.tile([128, G], BF16, tag="br_sb")
                nc.sync.dma_start(br_sb[:], brep[r])
                run_bufs[ri] = br_sb

            # phase-1 emission is BANK (quarter-tile, ~0.9us) granular: a full
            # tile is ~3.7us of PE work, which overflows a step's PE-idle tail
            # and displaces the next recurrent burst 1:1 in the PE FIFO.
            p1_tile_state = {}

            def emit_p1_bank(i, jb):
                ci, r = tile_info[i]
                ri = run_of_tile[i]
                if jb == 0:
                    load_run_weights(ri)
                    if i == runs[ri][0]:
                        load_run_weights(ri + 1)   # prefetch next run's bias
                        ensure_wf(r)
                        for rj in range(ri + 1, min(ri + 3, len(runs))):
                            ensure_wf(runs[rj][1])
                        if ri - 2 in run_bufs:
                            del run_bufs[ri - 2]
                    xt_sb = xt_pool.tile([128, KD, 128], BF16, tag="xt_sb")
                    nc.sync.dma_start(xt_sb[:], xt[i])
                    gxs = gxs_pool.tile([128, G], BF16, tag="gxs")
                    p1_tile_state[i] = (xt_sb, gxs)
                br_sb = run_bufs[ri]
                xt_sb, gxs = p1_tile_state[i]
                sl = slice(jb * 512, (jb + 1) * 512)
                ps_b = ps1_pool.tile([128, 512], F32, tag="ps1",
                                     name=f"ps1_{jb}")
                for k in range(KD):
                    nc.tensor.matmul(
                        ps_b[:], xt_sb[:, k, :], wf_res[r][:, k, sl],
                        start=(k == 0), stop=(k == KD - 1),
                    )
                # PSUM eviction with fused bias on the vector engine:
                # scalar-queue copies would delay the next step's gate
                # activations, which are chain-critical
                nc.vector.tensor_add(gxs[:, sl], ps_b[:], br_sb[:, sl])
                if jb == 3:
                    del p1_tile_state[i]
                    # gx store on the gpsimd (SWDGE) queue: keeps the sync
                    # queue a pure-load FIFO so weight prefetches are never
                    # stuck behind a store that waits on compute
                    wi = nc.gpsimd.dma_start(gx[i * 128:(i + 1) * 128, :],
                                             gxs[:])
                    p1_writes[ci].append(wi.ins)
                    p1_emitted[ci] += 1

            def emit_p1_tile(i):
                for jb in range(4):
                    emit_p1_bank(i, jb)

            # chunk 0 runs before the LSTM
            n_c0 = chunk_ntiles[0]
            for i in range(n_c0):
                emit_p1_tile(i)
            pending = [(i, jb) for i in range(n_c0, ntiles) for jb in range(4)]

            # deadline-aware pacing (in bank units): chunk ci must be
            # emitted a few steps before its first gather; spread the rest
            # evenly so the PE stays fed (and HAM stays warm) across as many
            # LSTM steps as possible.
            targets = []
            cum = 0
            for ci in range(1, nchunks):
                cum += 4 * chunk_ntiles[ci]
                targets.append((max(0, CHUNK_TS[ci] - 4), cum))
            targets.append((nsteps - 8, cum))   # spread any slack work
            p1_quota = [0] * nsteps
            emitted_plan = 0
            acc = 0.0
            for t in range(nsteps):
                rate = 0.0
                for d, need in targets:
                    if d >= t and need > emitted_plan + acc:
                        rate = max(rate, (need - emitted_plan - acc) / (d - t + 1))
                acc += rate
                n = int(acc)
                acc -= n
                p1_quota[t] = n
                emitted_plan += n

            # ---------------- phase 2: LSTM ------------------------------
            wh_sb = const_pool.tile([128, KD, G], BF16)
            nc.sync.dma_start(wh_sb[:], wh[:])
            idx_sb = const_pool.tile([128, nsteps], I32)
            nc.sync.dma_start(idx_sb[:], gidx[:])

            c_sb = st_pool.tile([128, D], BF16)
            tmp1 = st_pool.tile([128, D], BF16)
            tmp2 = st_pool.tile([128, D], BF16)

            ht_sb = None
            gxt_tiles = {}
            banks = {}

            def emit_gather(t):
                ci = chunk_of_t[t]
                # make sure the chunk's phase-1 tiles are all emitted
                while p1_emitted[ci] < chunk_ntiles[ci] and pending:
                    emit_p1_bank(*pending.pop(0))
                gxt = gx_pool.tile([128, G], BF16, tag="gxt")
                rows_end = chunk_rows_end[ci]
                gi = nc.gpsimd.indirect_dma_start(
                    out=gxt[:],
                    out_offset=None,
                    in_=gx[0:rows_end, :],
                    in_offset=bass.IndirectOffsetOnAxis(
                        ap=idx_sb[:, t:t + 1], axis=0
                    ),
                )
                # the tracker cannot see through the dynamic row offsets, so
                # order the gather after every write of its chunk explicitly
                for w in p1_writes[ci]:
                    add_dep_helper(gi.ins, w, reason="gather waits gx chunk")
                gxt_tiles[t] = gxt

            def emit_ident(t, jb):
                # first write of bank jb for step t: gates <- gx rows
                psb = ps2_pool.tile([128, 512], F32, tag="ps2")
                nc.tensor.matmul(
                    psb[:], ident_r[:],
                    gxt_tiles[t][:, jb * 512:(jb + 1) * 512],
                    start=True, stop=(t == 0),
                )
                banks[(t, jb)] = psb

            # bank processing order: g first so the c-chain overlaps later banks
            BORD = (2, 0, 1, 3)
            emit_gather(0)
            emit_gather(1)
            for jb in BORD:
                emit_ident(0, jb)
            for t in range(nsteps):
                bs = nsteps - t
                if t + 2 < nsteps:
                    emit_gather(t + 2)
                sif = act_pool.tile([128, 2 * D], BF16, tag="sif")
                tg = act_pool.tile([128, D], BF16, tag="tg")
                so = act_pool.tile([128, D], BF16, tag="so")
                act_of = {
                    2: (tg, 0, AF.Tanh),
                    0: (sif, 0, AF.Sigmoid),
                    1: (sif, D, AF.Sigmoid),
                    3: (so, 0, AF.Sigmoid),
                }
                H = D // 2
                # recurrent accumulation, bank-major so the chain-critical
                # gates (g, i, f) complete and activate as early as possible;
                # k 0,1 use half 0 of the transposed h (ready first), k 2,3
                # use half 1
                def rec(jb, ks):
                    sl = slice(jb * 512, (jb + 1) * 512)
                    psb = banks[(t, jb)]
                    for k in ks:
                        nc.tensor.matmul(
                            psb[:],
                            ht_sb[:, k * 128:(k + 1) * 128],
                            wh_sb[:, k, sl],
                            start=False,
                            stop=(k == KD - 1),
                        )

                def act(jb):
                    tile_, base, fn = act_of[jb]
                    nc.scalar.activation(
                        tile_[:, base:base + D], banks.pop((t, jb))[:], fn)

                def act_h(jb, hh):
                    # half-width activation: shortens the chain to the first
                    # c-update half (the second half trails in parallel)
                    tile_, base, fn = act_of[jb]
                    psb = banks[(t, jb)] if hh == 0 else banks.pop((t, jb))
                    nc.scalar.activation(
                        tile_[:, base + hh * H:base + (hh + 1) * H],
                        psb[:, hh * H:(hh + 1) * H], fn)

                if t > 0:
                    rec(2, (0, 1))
                    rec(0, (0, 1))
                    rec(2, (2, 3))
                    act(2)
                    rec(0, (2, 3))
                    act(0)
                    rec(1, (0, 1, 2, 3))
                    act_h(1, 0)
                    act_h(1, 1)
                    rec(3, (0, 1, 2, 3))
                    # o in halves: half 0 lands before tanh(c) half 0 in the
                    # ACT FIFO (a full-width o would delay it by ~290ns)
                    act_h(3, 0)
                else:
                    for jb in BORD:
                        act(jb)
                # inject next step's gx only after the chain-critical matmuls
                # above, so the idents don't displace them in the PE queue
                if t + 1 < nsteps:
                    for jb in BORD:
                        emit_ident(t + 1, jb)
                # slot A for phase-1 banks: right after the idents there is
                # ~1us of PE-idle before the transposes become ready, so one
                # bank (~0.9us) rides free here
                slot_a = 1 if (p1_quota[t] >= 1 and pending) else 0
                if slot_a:
                    emit_p1_bank(*pending.pop(0))

                # c update in halves (all on VectorE: a concurrent GpSimd
                # elementwise op would contend for the shared SBUF port);
                # half 0 completes right after act_f half 0 so tanh(c) and
                # the h/transpose chain start ~0.5us earlier
                if t == 0:
                    nc.vector.tensor_tensor(
                        c_sb[:], sif[:, 0:D], tg[:], mybir.AluOpType.mult
                    )
                else:
                    nc.vector.tensor_tensor(
                        tmp2[:], sif[:, 0:D], tg[:], mybir.AluOpType.mult
                    )
                    for hh in range(2):
                        sl = slice(hh * H, (hh + 1) * H)
                        slf = slice(D + hh * H, D + (hh + 1) * H)
                        nc.vector.tensor_tensor(
                            tmp1[:, sl], sif[:, slf], c_sb[:, sl],
                            mybir.AluOpType.mult
                        )
                        nc.vector.tensor_add(c_sb[:, sl], tmp1[:, sl],
                                             tmp2[:, sl])
                tc_sb = act_pool.tile([128, D], BF16, tag="tc_sb")
                # tanh(c) / h / transpose / cast in halves so the next step's
                # first recurrent matmuls (k=0,1) start as soon as half 0 of
                # the transposed h is ready
                h_sb = h_pool.tile([128, D], BF16, tag="h_sb")
                if t < nsteps - 1:
                    trp = tr_pool.tile([128, D], BF16, tag="trp")
                    new_ht = ht_pool.tile([128, D], BF16, tag="ht_sb")
                for hh in range(2):
                    sl = slice(hh * H, (hh + 1) * H)
                    nc.scalar.activation(tc_sb[:, sl], c_sb[:, sl], AF.Tanh)
                    if hh == 0 and t > 0:
                        act_h(3, 1)   # o half 1, off the h-half-0 path
                    nc.vector.tensor_tensor(
                        h_sb[:, sl], so[:, sl], tc_sb[:, sl],
                        mybir.AluOpType.mult,
                    )
                    if t < nsteps - 1:
                        for k in (2 * hh, 2 * hh + 1):
                            nc.tensor.transpose(
                                trp[:, k * 128:(k + 1) * 128],
                                h_sb[:, k * 128:(k + 1) * 128],
                                ident_r[:],
                            )
                        nc.vector.tensor_copy(new_ht[:, sl], trp[:, sl])
                if t < nsteps - 1:
                    ht_sb = new_ht
                # stream out this step's hidden states (packed rows); scalar
                # (HWDGE) queue so the sync queue stays a pure-load FIFO
                nc.scalar.dma_start(
                    out[int(loc_off[t]):int(loc_off[t]) + bs, :], h_sb[:bs, :]
                )
                # slot B: remaining phase-1 banks for this step (emitted last
                # so their vector-engine evictions queue behind this step's
                # chain-critical c/h ops, not in front of them)
                for _ in range(p1_quota[t] - slot_a):
                    if pending:
                        emit_p1_bank(*pending.pop(0))
    return nc


# ---------------------------------------------------------------------------
# Host-side data marshaling
# ---------------------------------------------------------------------------
def _expected_layout():
    lengths = T - np.arange(B) // NCORES
    batch_sizes = np.array([(lengths > t).sum() for t in range(T)], dtype=np.int32)
    time_idx = np.concatenate(
        [np.full(bs, t, np.int32) for t, bs in enumerate(batch_sizes)]
    )
    batch_idx = np.concatenate(
        [np.arange(bs, dtype=np.int32) for bs in batch_sizes]
    )
    return batch_sizes, time_idx, batch_idx


def _numpy_reference(embed, W_rel, b_rel, W_ih, W_hh, b_ih, b_hh,
                     nodes, rels, time_idx, batch_idx, batch_sizes):
    """Pure-numpy fallback (only used if the packed layout differs from the
    hardcoded one)."""
    n_steps = int(batch_sizes.shape[0])
    max_bs = int(batch_sizes.max())
    x = embed[nodes]
    y = np.zeros_like(x)
    for r in range(W_rel.shape[0]):
        m = rels == r
        y[m] = x[m] @ W_rel[r].T + b_rel[r]
    d = x.shape[-1]
    xp = np.zeros((n_steps, max_bs, d), x.dtype)
    mask = np.zeros((n_steps, max_bs), bool)
    xp[time_idx, batch_idx] = y
    mask[time_idx, batch_idx] = True
    bias = b_ih + b_hh

    def sig(v):
        return 1.0 / (1.0 + np.exp(-v))

    h = np.zeros((max_bs, d), x.dtype)
    c = np.zeros((max_bs, d), x.dtype)
    hs = np.zeros((n_steps, max_bs, d), x.dtype)
    for t in range(n_steps):
        gates = xp[t] @ W_ih.T + h @ W_hh.T + bias
        i, f, g, o = np.split(gates, 4, axis=-1)
        c_new = sig(f) * c + sig(i) * np.tanh(g)
        h_new = sig(o) * np.tanh(c_new)
        m = mask[t][:, None]
        h = np.where(m, h_new, h)
        c = np.where(m, c_new, c)
        hs[t] = h
    return hs[time_idx, batch_idx]


def _prepare_host(inputs, nsteps=T):
    """Build per-core device input dicts + the output unshard map."""
    embed = np.asarray(inputs["embed"], np.float32)
    W_rel = np.asarray(inputs["W_rel"], np.float32)
    b_rel = np.asarray(inputs["b_rel"], np.float32)
    W_ih = np.asarray(inputs["W_ih"], np.float32)
    W_hh = np.asarray(inputs["W_hh"], np.float32)
    b_ih = np.asarray(inputs["b_ih"], np.float32)
    b_hh = np.asarray(inputs["b_hh"], np.float32)
    nodes = np.asarray(inputs["nodes"])
    rels = np.asarray(inputs["rels"])

    nchunks = len(CHUNK_TS) - 1
    nloc = nsteps * (nsteps + 1) // 2

    # fused weights & biases (float64 for accuracy, cast down)
    Wfuse = (W_ih.astype(np.float64) @ W_rel.astype(np.float64))
    Wfuse = Wfuse.astype(np.float32)            # [R, G, D]
    btot = (W_ih.astype(np.float64) @ b_rel.astype(np.float64).T).T \
        + (b_ih + b_hh).astype(np.float64)      # [R, G]
    btot = btot.astype(np.float32)

    wf_host = np.ascontiguousarray(
        Wfuse.transpose(0, 2, 1).reshape(R, KD, 128, G).transpose(0, 2, 1, 3)
    ).astype(NPBF16)                             # [R, 128(dk), KD, G]
    wh_host = np.ascontiguousarray(
        W_hh.T.reshape(KD, 128, G).transpose(1, 0, 2)
    ).astype(NPBF16)                             # [128(dk), KD, G]
    bt_host = np.ascontiguousarray(btot[:, None, :]).astype(NPBF16)  # [R,1,G]
    brep_host = np.ascontiguousarray(
        np.broadcast_to(btot[:, None, :], (R, 128, G))
    ).astype(NPBF16)                             # [R, 128, G]

    # local token enumeration (identical structure for every core)
    t_arr = np.concatenate(
        [np.full(nsteps - t, t, np.int64) for t in range(nsteps)]
    )
    j_arr = np.concatenate(
        [np.arange(nsteps - t, dtype=np.int64) for t in range(nsteps)]
    )
    gbs = NCORES * (nsteps - np.arange(nsteps, dtype=np.int64))
    goff = np.concatenate([[0], np.cumsum(gbs)])

    chunk_of_t = np.zeros(nsteps, np.int64)
    for ci in range(nchunks):
        chunk_of_t[CHUNK_TS[ci]:CHUNK_TS[ci + 1]] = ci
    ch_loc = chunk_of_t[t_arr]

    # per-core per-(chunk,rel) token counts -> exact shared tile budgets
    rel_by_core = []
    counts = np.zeros((NCORES, nchunks, R), np.int64)
    for core in range(NCORES):
        grow = goff[t_arr] + NCORES * j_arr + core
        rel_loc = rels[grow].astype(np.int64)
        rel_by_core.append((grow, rel_loc))
        np.add.at(counts[core], (ch_loc, rel_loc), 1)
    tiles_cr = [
        [int(np.ceil(counts[:, ci, r].max() / 128)) if counts[:, ci, r].max() > 0
         else 0 for r in range(R)]
        for ci in range(nchunks)
    ]

    # segment bases (must mirror build_program's tile order)
    seg_base = {}
    acc_tiles = 0
    for ci in range(nchunks):
        for r in range(R):
            seg_base[(ci, r)] = acc_tiles * 128
            acc_tiles += tiles_cr[ci][r]
    ntiles = acc_tiles

    in_maps = []
    for core in range(NCORES):
        grow, rel_loc = rel_by_core[core]
        node_loc = nodes[grow]

        order = np.lexsort((j_arr, t_arr, rel_loc, ch_loc))
        key = ch_loc[order] * R + rel_loc[order]
        cnt = np.bincount(key, minlength=nchunks * R)
        q = np.concatenate([np.arange(c) for c in cnt])
        base_sorted = np.array(
            [seg_base[(k // R, k % R)] for k in key], np.int64
        )
        prow_sorted = base_sorted + q
        prow = np.empty(nloc, np.int64)
        prow[order] = prow_sorted

        gidx_host = np.zeros((128, nsteps), np.int32)
        gidx_host[j_arr, t_arr] = prow

        Xp = np.zeros((ntiles * 128, D), np.float32)
        Xp[prow] = embed[node_loc]
        xt_host = np.ascontiguousarray(
            Xp.reshape(ntiles, 128, KD, 128).transpose(0, 3, 2, 1)
        ).astype(NPBF16)                         # [NT, 128(dk), KD, 128(tok)]

        in_maps.append({
            "xt": xt_host,
            "wf": wf_host,
            "wh": wh_host,
            "bt": bt_host,
            "brep": brep_host,
            "gidx": gidx_host,
        })

    unshard = {
        "t_arr": t_arr, "j_arr": j_arr, "goff": goff,
        "nloc": nloc,
    }
    return in_maps, unshard, tiles_cr


def kernel(**inputs):
    global LAST_RESULTS
    import os

    # Verify the packed layout matches the hardcoded structure.
    bs_exp, ti_exp, bi_exp = _expected_layout()
    ok = (
        np.array_equal(np.asarray(inputs["batch_sizes"]), bs_exp)
        and np.array_equal(np.asarray(inputs["time_idx"]), ti_exp)
        and np.array_equal(np.asarray(inputs["batch_idx"]), bi_exp)
        and np.asarray(inputs["embed"]).shape == (50000, D)
    )
    if not ok:
        return _numpy_reference(**{k: np.asarray(v) for k, v in inputs.items()})

    in_maps, unshard, tiles_cr = _prepare_host(inputs)

    nc = build_program(tiles_cr)
    trace = bool(os.environ.get("KERNEL_TRACE"))
    res = bass_utils.run_bass_kernel_spmd(
        nc, in_maps, core_ids=list(range(NCORES)), trace=trace,
    )
    LAST_RESULTS = res

    t_arr = unshard["t_arr"]
    j_arr = unshard["j_arr"]
    goff = unshard["goff"]
    out_full = np.zeros((len(np.asarray(inputs["time_idx"])), D), np.float32)
    for core in range(NCORES):
        grow = goff[t_arr] + NCORES * j_arr + core
        out_full[grow] = np.asarray(res.results[core]["out"], np.float32)
    return out_full


# revision 50
# speedup vs baseline: 1.0135x; 1.0108x over previous
"""Trainium2 Bass kernel for nn_Evolution_4664334483942 (moe_routing).

Model: per-token relation-specific linear (MoE dispatch) feeding a packed
variable-length-sequence LSTM.

Strategy (data-parallel over sequences, 8 cores, no collectives):
  - Global batch b (0..1023) assigned to core b % 8.  Every core then holds
    128 sequences with lengths 128,127,...,1 (identical structure on every
    core), 8256 tokens each.
  - Host folds W_ih @ W_rel[r].T into per-relation fused weights so the MoE
    projection and the LSTM input projection collapse into ONE GEMM:
        gx[n] = x[n] @ Wfuse[rel_n].T + (W_ih b_rel[rel_n] + b_ih + b_hh)
  - Phase 1 (device): dense bf16 GEMM over rel-sorted 128-token tiles,
    writing gx (bf16) to DRAM.  The per-(chunk,rel) bias is injected with a
    rank-1 ones x bias matmul (start=True) so no vector-engine work is
    needed; PSUM->SBUF copies go on the scalar + vector engines.
  - Tokens are split into TIME CHUNKS; chunk 0 is computed before the LSTM
    and later chunks are interleaved into the LSTM's tensor-engine gaps
    (1 tile/step + forced drain at chunk boundaries).  This keeps the PE
    busy through the per-step scalar/vector tail — which also keeps the
    HAM clock-gate warm (2.4 GHz) instead of re-throttling every step.
  - Phase 2 (device): 128 sequential LSTM steps.  Each step gathers its
    gx rows via indirect DMA (per-core index table = data, so the SPMD
    instruction stream stays core-independent), feeds them into the gates
    PSUM via an identity matmul, accumulates h @ W_hh.T on top (bf16
    operands), applies sigmoid/tanh on ScalarE, c/h updates on VectorE
    (fp32 state), PE-transposes bf16 h for the next step, and streams h
    out to DRAM in bf16.
"""

import numpy as np
import ml_dtypes

import concourse.bass as bass
import concourse.mybir as mybir
import concourse.tile as tile
from concourse import bass_utils
from concourse.masks import make_identity
from bass_rust import add_dep_helper
from concourse.vector_clock import ScopedClock

F32 = mybir.dt.float32
BF16 = mybir.dt.bfloat16
I32 = mybir.dt.int32
AF = mybir.ActivationFunctionType
NPBF16 = ml_dtypes.bfloat16

NCORES = 8

# Problem constants (hardcoded; kernel.py must be self-contained).
D = 512          # hidden dim
R = 8            # relations
T = 128          # max sequence length / LSTM steps
B = 1024         # global sequences
KD = D // 128    # contraction k-tiles
G = 4 * D        # gate width (2048)
NJB = G // 512   # psum banks for gates

# phase-1 time-chunk boundaries.  Chunk 0 runs before the LSTM; later
# chunks interleave into the LSTM's tensor-engine gaps (step t only ever
# gathers rows of its own chunk).  Tile counts per (chunk, rel) are
# computed exactly from the data at runtime (max over cores).
CHUNK_TS = (0, 32, 64, 96, 128)

# Results of the last device run (test harness reads exec_time_ns from here).
LAST_RESULTS = None


# ---------------------------------------------------------------------------
# Walrus in this toolchain accepts only ONE sync-wait command per instruction;
# Tile's wait assignment can attach several.  Peel the extras onto same-engine
# NOPs placed immediately before the offending instruction.
# ---------------------------------------------------------------------------
def _split_waits_in_list(nc, insts, max_waits=1):
    out = []
    for inst in insts:
        si = inst.sync_info
        if si is not None and si.on_wait is not None and len(si.on_wait) > max_waits:
            waits = list(si.on_wait)
            for w in waits[max_waits:]:
                nop = mybir.InstNoOp(
                    name=nc.get_next_instruction_name(), ins=[], outs=[],
                )
                nop.engine = inst.engine
                nop.sync_info = mybir.SyncInfo(on_wait=[w], on_update=[])
                out.append(nop)
            inst.sync_info = mybir.SyncInfo(
                on_wait=waits[:max_waits], on_update=list(si.on_update or [])
            )
        out.append(inst)
    return out


class PatchedTileContext(tile.TileContext):
    def _lower_ordered_insts(self, ordered):
        for bb_name in list(ordered.keys()):
            ordered[bb_name] = _split_waits_in_list(self.nc, ordered[bb_name])
        super()._lower_ordered_insts(ordered)

    def _drain_and_barrier(self, tick_clock, wait_clock):
        nop_inst = self.nc.sync.nop()
        wait_clock.add_sem_waits(
            nop_inst.ins, ScopedClock({None: tick_clock.global_clock})
        )
        si = nop_inst.ins.sync_info
        if si is not None and si.on_wait and len(si.on_wait) > 1:
            waits = list(si.on_wait)
            nop_inst.ins.sync_info = mybir.SyncInfo(
                on_wait=[waits[0]], on_update=list(si.on_update or [])
            )
            for w in waits[1:]:
                extra = self.nc.sync.nop()
                extra.ins.sync_info = mybir.SyncInfo(on_wait=[w], on_update=[])
        self.nc.sync.drain()
        self.nc.all_engine_barrier()
        assert self.sems is not None
        popped = self.nc._tile_sem_poison_stack.pop()
        assert popped is self._sem_poison
        self.nc.clear_and_free_semaphores(list(self.sems.allocated().values()))
        self.nc.all_engine_barrier()


# ---------------------------------------------------------------------------
# Device program (core-independent instruction stream; per-core variation is
# carried entirely by input data: xt tile contents and the gather index table)
# ---------------------------------------------------------------------------
def build_program(tiles_cr, nsteps=T):
    """tiles_cr: list (per chunk) of per-rel tile counts [R]."""
    nchunks = len(tiles_cr)
    assert nchunks == len(CHUNK_TS) - 1

    # physical tile order: chunk-major, then rel, then tile
    tile_info = []          # (chunk_idx, rel)
    for ci in range(nchunks):
        for r in range(R):
            for _ in range(tiles_cr[ci][r]):
                tile_info.append((ci, r))
    ntiles = len(tile_info)
    nrows = ntiles * 128
    nloc = nsteps * (nsteps + 1) // 2

    chunk_ntiles = [sum(tiles_cr[ci]) for ci in range(nchunks)]
    chunk_rows_end = []
    acc = 0
    for ci in range(nchunks):
        acc += chunk_ntiles[ci] * 128
        chunk_rows_end.append(acc)
    chunk_of_t = {}
    for ci in range(nchunks):
        for t in range(CHUNK_TS[ci], CHUNK_TS[ci + 1]):
            chunk_of_t[t] = ci

    nc = bass.Bass(target_bir_lowering=False, debug=False, trn_type="TRN2")

    xt = nc.dram_tensor("xt", [ntiles, 128, KD, 128], BF16, kind="ExternalInput").ap()
    wf = nc.dram_tensor("wf", [R, 128, KD, G], BF16, kind="ExternalInput").ap()
    wh = nc.dram_tensor("wh", [128, KD, G], BF16, kind="ExternalInput").ap()
    bt = nc.dram_tensor("bt", [R, 1, G], BF16, kind="ExternalInput").ap()
    brep = nc.dram_tensor("brep", [R, 128, G], BF16, kind="ExternalInput").ap()
    gidx = nc.dram_tensor("gidx", [128, nsteps], I32, kind="ExternalInput").ap()
    out = nc.dram_tensor("out", [nloc, D], BF16, kind="ExternalOutput").ap()
    gx = nc.dram_tensor("gx", [nrows, G], BF16).ap()

    loc_bs = [nsteps - t for t in range(nsteps)]
    loc_off = np.concatenate([[0], np.cumsum(loc_bs)]).astype(int)

    with PatchedTileContext(nc) as tc:
        with tc.tile_pool(name="p1_xt", bufs=5) as xt_pool, \
             tc.tile_pool(name="p1_br", bufs=3) as br_pool, \
             tc.tile_pool(name="p1_gx", bufs=4) as gxs_pool, \
             tc.tile_pool(name="p1_ps", bufs=3, space="PSUM") as ps1_pool, \
             tc.tile_pool(name="p2_const", bufs=1) as const_pool, \
             tc.tile_pool(name="p2_gx", bufs=3) as gx_pool, \
             tc.tile_pool(name="p2_act", bufs=2) as act_pool, \
             tc.tile_pool(name="p2_st", bufs=1) as st_pool, \
             tc.tile_pool(name="p2_h", bufs=2) as h_pool, \
             tc.tile_pool(name="p2_ht", bufs=2) as ht_pool, \
             tc.tile_pool(name="p2_ps", bufs=4, space="PSUM") as ps2_pool, \
             tc.tile_pool(name="p2_tr", bufs=1, space="PSUM") as tr_pool:

            # identity (bf16): stationary for gx-inject matmuls + transposes
            ident_r = const_pool.tile([128, 128], BF16)
            make_identity(nc, ident_r[:])

            # the fused per-relation weights stay fully resident in SBUF
            # (16.8 MB) — reloading 2 MB per rel-run saturates HBM on top of
            # the xt/gx streams and stalls the PE at every run boundary.
            # One tile per relation, loaded lazily two rel-runs ahead so the
            # 16.8 MB doesn't all compete with the first tiles' xt/gx traffic.
            wf_res = [
                const_pool.tile([128, KD, G], BF16, name=f"wf_res{r}")
                for r in range(R)
            ]
            wf_loaded = set()

            def ensure_wf(r):
                if r not in wf_loaded:
                    nc.scalar.dma_start(wf_res[r][:], wf[r])
                    wf_loaded.add(r)

            # ---------------- phase-1 tile emitter -----------------------
            # rel-runs (maximal groups of consecutive same-rel tiles) so the
            # next run's 2 MB weight DMA can be prefetched a full run early
            runs = []
            run_of_tile = []
            for i, (ci, r) in enumerate(tile_info):
                if not runs or runs[-1][1] != r:
                    runs.append((i, r))
                run_of_tile.append(len(runs) - 1)
            run_bufs = {}
            p1_writes = [[] for _ in range(nchunks)]
            p1_emitted = [0 for _ in range(nchunks)]

            def load_run_weights(ri):
                if ri >= len(runs) or ri in run_bufs:
                    return
                # sync ring (pure loads) — the scalar ring is busy streaming
                # the 16.8 MB resident wf at startup and would delay this,
                # which in turn would block PSUM eviction of the first tiles
                r = runs[ri][1]
                br_sb = br_pool.tile([128, G], BF16, tag="br_sb")
                nc.sync.dma_start(br_sb[:], brep[r])
                run_bufs[ri] = br_sb

            # phase-1 emission is BANK (quarter-tile, ~0.9us) granular: a full
            # tile is ~3.7us of PE work, which overflows a step's PE-idle tail
            # and displaces the next recurrent burst 1:1 in the PE FIFO.
            p1_tile_state = {}

            def emit_p1_bank(i, jb):
                ci, r = tile_info[i]
                ri = run_of_tile[i]
                if jb == 0:
                    load_run_weights(ri)
                    if i == runs[ri][0]:
                        load_run_weights(ri + 1)   # prefetch next run's bias
                        ensure_wf(r)
                        for rj in range(ri + 1, min(ri + 3, len(runs))):
                            ensure_wf(runs[rj][1])
                        if ri - 2 in run_bufs:
                            del run_bufs[ri - 2]
                    xt_sb = xt_pool.tile([128, KD, 128], BF16, tag="xt_sb")
                    nc.sync.dma_start(xt_sb[:], xt[i])
                    gxs = gxs_pool.tile([128, G], BF16, tag="gxs")
                    p1_tile_state[i] = (xt_sb, gxs)
                br_sb = run_bufs[ri]
                xt_sb, gxs = p1_tile_state[i]
                sl = slice(jb * 512, (jb + 1) * 512)
                ps_b = ps1_pool.tile([128, 512], F32, tag="ps1",
                                     name=f"ps1_{jb}")
                for k in range(KD):
                    nc.tensor.matmul(
                        ps_b[:], xt_sb[:, k, :], wf_res[r][:, k, sl],
                        start=(k == 0), stop=(k == KD - 1),
                    )
                # PSUM eviction with fused bias on the vector engine:
                # scalar-queue copies would delay the next step's gate
                # activations, which are chain-critical
                nc.vector.tensor_add(gxs[:, sl], ps_b[:], br_sb[:, sl])
                if jb == 3:
                    del p1_tile_state[i]
                    # gx store on the gpsimd (SWDGE) queue: keeps the sync
                    # queue a pure-load FIFO so weight prefetches are never
                    # stuck behind a store that waits on compute
                    wi = nc.gpsimd.dma_start(gx[i * 128:(i + 1) * 128, :],
                                             gxs[:])
                    p1_writes[ci].append(wi.ins)
                    p1_emitted[ci] += 1

            def emit_p1_tile(i):
                for jb in range(4):
                    emit_p1_bank(i, jb)

            # chunk 0 runs before the LSTM
            n_c0 = chunk_ntiles[0]
            for i in range(n_c0):
                emit_p1_tile(i)
            pending = [(i, jb) for i in range(n_c0, ntiles) for jb in range(4)]

            # deadline-aware pacing (in bank units): chunk ci must be
            # emitted a few steps before its first gather; spread the rest
            # evenly so the PE stays fed (and HAM stays warm) across as many
            # LSTM steps as possible.
            targets = []
            cum = 0
            for ci in range(1, nchunks):
                cum += 4 * chunk_ntiles[ci]
                targets.append((max(0, CHUNK_TS[ci] - 4), cum))
            targets.append((nsteps - 8, cum))   # spread any slack work
            p1_quota = [0] * nsteps
            emitted_plan = 0
            acc = 0.0
            for t in range(nsteps):
                rate = 0.0
                for d, need in targets:
                    if d >= t and need > emitted_plan + acc:
                        rate = max(rate, (need - emitted_plan - acc) / (d - t + 1))
                acc += rate
                n = int(acc)
                acc -= n
                p1_quota[t] = n
                emitted_plan += n

            # ---------------- phase 2: LSTM ------------------------------
            wh_sb = const_pool.tile([128, KD, G], BF16)
            nc.sync.dma_start(wh_sb[:], wh[:])
            idx_sb = const_pool.tile([128, nsteps], I32)
            nc.sync.dma_start(idx_sb[:], gidx[:])

            c_sb = st_pool.tile([128, D], BF16)
            tmp1 = st_pool.tile([128, D], BF16)
            tmp2 = st_pool.tile([128, D], BF16)

            ht_sb = None
            gxt_tiles = {}
            banks = {}

            def emit_gather(t):
                ci = chunk_of_t[t]
                # make sure the chunk's phase-1 tiles are all emitted
                while p1_emitted[ci] < chunk_ntiles[ci] and pending:
                    emit_p1_bank(*pending.pop(0))
                gxt = gx_pool.tile([128, G], BF16, tag="gxt")
                rows_end = chunk_rows_end[ci]
                gi = nc.gpsimd.indirect_dma_start(
                    out=gxt[:],
                    out_offset=None,
                    in_=gx[0:rows_end, :],
                    in_offset=bass.IndirectOffsetOnAxis(
                        ap=idx_sb[:, t:t + 1], axis=0
                    ),
                )
                # the tracker cannot see through the dynamic row offsets, so
                # order the gather after every write of its chunk explicitly
                for w in p1_writes[ci]:
                    add_dep_helper(gi.ins, w, reason="gather waits gx chunk")
                gxt_tiles[t] = gxt

            def emit_ident(t, jb):
                # first write of bank jb for step t: gates <- gx rows
                psb = ps2_pool.tile([128, 512], F32, tag="ps2")
                nc.tensor.matmul(
                    psb[:], ident_r[:],
                    gxt_tiles[t][:, jb * 512:(jb + 1) * 512],
                    start=True, stop=(t == 0),
                )
                banks[(t, jb)] = psb

            # bank processing order: g first so the c-chain overlaps later banks
            BORD = (2, 0, 1, 3)
            emit_gather(0)
            emit_gather(1)
            for jb in BORD:
                emit_ident(0, jb)
            for t in range(nsteps):
                bs = nsteps - t
                if t + 2 < nsteps:
                    emit_gather(t + 2)
                sif = act_pool.tile([128, 2 * D], BF16, tag="sif")
                tg = act_pool.tile([128, D], BF16, tag="tg")
                so = act_pool.tile([128, D], BF16, tag="so")
                act_of = {
                    2: (tg, 0, AF.Tanh),
                    0: (sif, 0, AF.Sigmoid),
                    1: (sif, D, AF.Sigmoid),
                    3: (so, 0, AF.Sigmoid),
                }
                H = D // 2
                # recurrent accumulation, bank-major so the chain-critical
                # gates (g, i, f) complete and activate as early as possible;
                # k 0,1 use half 0 of the transposed h (ready first), k 2,3
                # use half 1
                def rec(jb, ks):
                    sl = slice(jb * 512, (jb + 1) * 512)
                    psb = banks[(t, jb)]
                    for k in ks:
                        nc.tensor.matmul(
                            psb[:],
                            ht_sb[:, k * 128:(k + 1) * 128],
                            wh_sb[:, k, sl],
                            start=False,
                            stop=(k == KD - 1),
                        )

                def act(jb):
                    tile_, base, fn = act_of[jb]
                    nc.scalar.activation(
                        tile_[:, base:base + D], banks.pop((t, jb))[:], fn)

                def act_h(jb, hh):
                    # half-width activation: shortens the chain to the first
                    # c-update half (the second half trails in parallel)
                    tile_, base, fn = act_of[jb]
                    psb = banks[(t, jb)] if hh == 0 else banks.pop((t, jb))
                    nc.scalar.activation(
                        tile_[:, base + hh * H:base + (hh + 1) * H],
                        psb[:, hh * H:(hh + 1) * H], fn)

                if t > 0:
                    rec(2, (0, 1))
                    rec(0, (0, 1))
                    rec(2, (2, 3))
                    act(2)
                    rec(0, (2, 3))
                    act(0)
                    rec(1, (0, 1, 2, 3))
                    act_h(1, 0)
                    act_h(1, 1)
                    rec(3, (0, 1, 2, 3))
                    # o in halves: half 0 lands before tanh(c) half 0 in the
                    # ACT FIFO (a full-width o would delay it by ~290ns)
                    act_h(3, 0)
                else:
                    for jb in BORD:
                        act(jb)
                # inject next step's gx only after the chain-critical matmuls
                # above, so the idents don't displace them in the PE queue
                if t + 1 < nsteps:
                    for jb in BORD:
                        emit_ident(t + 1, jb)
                # slot A for phase-1 banks: right after the idents there is
                # ~1us of PE-idle before the transposes become ready, so one
                # bank (~0.9us) rides free here
                slot_a = 1 if (p1_quota[t] >= 1 and pending) else 0
                if slot_a:
                    emit_p1_bank(*pending.pop(0))

                # c update in halves (all on VectorE: a concurrent GpSimd
                # elementwise op would contend for the shared SBUF port);
                # half 0 completes right after act_f half 0 so tanh(c) and
                # the h/transpose chain start ~0.5us earlier
                if t == 0:
                    nc.vector.tensor_tensor(
                        c_sb[:], sif[:, 0:D], tg[:], mybir.AluOpType.mult
                    )
                else:
                    for hh in range(2):
                        sl = slice(hh * H, (hh + 1) * H)
                        slf = slice(D + hh * H, D + (hh + 1) * H)
                        nc.vector.tensor_tensor(
                            tmp2[:, sl], sif[:, sl], tg[:, sl],
                            mybir.AluOpType.mult
                        )
                        nc.vector.tensor_tensor(
                            tmp1[:, sl], sif[:, slf], c_sb[:, sl],
                            mybir.AluOpType.mult
                        )
                        nc.vector.tensor_add(c_sb[:, sl], tmp1[:, sl],
                                             tmp2[:, sl])
                tc_sb = act_pool.tile([128, D], BF16, tag="tc_sb")
                # tanh(c) / h / transpose / cast in halves so the next step's
                # first recurrent matmuls (k=0,1) start as soon as half 0 of
                # the transposed h is ready
                h_sb = h_pool.tile([128, D], BF16, tag="h_sb")
                if t < nsteps - 1:
                    trp = tr_pool.tile([128, D], BF16, tag="trp")
                    new_ht = ht_pool.tile([128, D], BF16, tag="ht_sb")
                for hh in range(2):
                    sl = slice(hh * H, (hh + 1) * H)
                    nc.scalar.activation(tc_sb[:, sl], c_sb[:, sl], AF.Tanh)
                    if hh == 0 and t > 0:
                        act_h(3, 1)   # o half 1, off the h-half-0 path
                    nc.vector.tensor_tensor(
                        h_sb[:, sl], so[:, sl], tc_sb[:, sl],
                        mybir.AluOpType.mult,
                    )
                    if t < nsteps - 1:
                        for k in (2 * hh, 2 * hh + 1):
                            nc.tensor.transpose(
                                trp[:, k * 128:(k + 1) * 128],
                                h_sb[:, k * 128:(k + 1) * 128],
                                ident_r[:],
                            )
                        nc.vector.tensor_copy(new_ht[:, sl], trp[:, sl])
                if t < nsteps - 1:
                    ht_sb = new_ht
                # stream out this step's hidden states (packed rows); scalar
                # (HWDGE) queue so the sync queue stays a pure-load FIFO
                nc.scalar.dma_start(
                    out[int(loc_off[t]):int(loc_off[t]) + bs, :], h_sb[:bs, :]
                )
                # slot B: remaining phase-1 banks for this step (emitted last
                # so their vector-engine evictions queue behind this step's
                # chain-critical c/h ops, not in front of them)
                for _ in range(p1_quota[t] - slot_a):
                    if pending:
                        emit_p1_bank(*pending.pop(0))
    return nc


# ---------------------------------------------------------------------------
# Host-side data marshaling
# ---------------------------------------------------------------------------
def _expected_layout():
    lengths = T - np.arange(B) // NCORES
    batch_sizes = np.array([(lengths > t).sum() for t in range(T)], dtype=np.int32)
    time_idx = np.concatenate(
        [np.full(bs, t, np.int32) for t, bs in enumerate(batch_sizes)]
    )
    batch_idx = np.concatenate(
        [np.arange(bs, dtype=np.int32) for bs in batch_sizes]
    )
    return batch_sizes, time_idx, batch_idx


def _numpy_reference(embed, W_rel, b_rel, W_ih, W_hh, b_ih, b_hh,
                     nodes, rels, time_idx, batch_idx, batch_sizes):
    """Pure-numpy fallback (only used if the packed layout differs from the
    hardcoded one)."""
    n_steps = int(batch_sizes.shape[0])
    max_bs = int(batch_sizes.max())
    x = embed[nodes]
    y = np.zeros_like(x)
    for r in range(W_rel.shape[0]):
        m = rels == r
        y[m] = x[m] @ W_rel[r].T + b_rel[r]
    d = x.shape[-1]
    xp = np.zeros((n_steps, max_bs, d), x.dtype)
    mask = np.zeros((n_steps, max_bs), bool)
    xp[time_idx, batch_idx] = y
    mask[time_idx, batch_idx] = True
    bias = b_ih + b_hh

    def sig(v):
        return 1.0 / (1.0 + np.exp(-v))

    h = np.zeros((max_bs, d), x.dtype)
    c = np.zeros((max_bs, d), x.dtype)
    hs = np.zeros((n_steps, max_bs, d), x.dtype)
    for t in range(n_steps):
        gates = xp[t] @ W_ih.T + h @ W_hh.T + bias
        i, f, g, o = np.split(gates, 4, axis=-1)
        c_new = sig(f) * c + sig(i) * np.tanh(g)
        h_new = sig(o) * np.tanh(c_new)
        m = mask[t][:, None]
        h = np.where(m, h_new, h)
        c = np.where(m, c_new, c)
        hs[t] = h
    return hs[time_idx, batch_idx]


def _prepare_host(inputs, nsteps=T):
    """Build per-core device input dicts + the output unshard map."""
    embed = np.asarray(inputs["embed"], np.float32)
    W_rel = np.asarray(inputs["W_rel"], np.float32)
    b_rel = np.asarray(inputs["b_rel"], np.float32)
    W_ih = np.asarray(inputs["W_ih"], np.float32)
    W_hh = np.asarray(inputs["W_hh"], np.float32)
    b_ih = np.asarray(inputs["b_ih"], np.float32)
    b_hh = np.asarray(inputs["b_hh"], np.float32)
    nodes = np.asarray(inputs["nodes"])
    rels = np.asarray(inputs["rels"])

    nchunks = len(CHUNK_TS) - 1
    nloc = nsteps * (nsteps + 1) // 2

    # fused weights & biases (float64 for accuracy, cast down)
    Wfuse = (W_ih.astype(np.float64) @ W_rel.astype(np.float64))
    Wfuse = Wfuse.astype(np.float32)            # [R, G, D]
    btot = (W_ih.astype(np.float64) @ b_rel.astype(np.float64).T).T \
        + (b_ih + b_hh).astype(np.float64)      # [R, G]
    btot = btot.astype(np.float32)

    wf_host = np.ascontiguousarray(
        Wfuse.transpose(0, 2, 1).reshape(R, KD, 128, G).transpose(0, 2, 1, 3)
    ).astype(NPBF16)                             # [R, 128(dk), KD, G]
    wh_host = np.ascontiguousarray(
        W_hh.T.reshape(KD, 128, G).transpose(1, 0, 2)
    ).astype(NPBF16)                             # [128(dk), KD, G]
    bt_host = np.ascontiguousarray(btot[:, None, :]).astype(NPBF16)  # [R,1,G]
    brep_host = np.ascontiguousarray(
        np.broadcast_to(btot[:, None, :], (R, 128, G))
    ).astype(NPBF16)                             # [R, 128, G]

    # local token enumeration (identical structure for every core)
    t_arr = np.concatenate(
        [np.full(nsteps - t, t, np.int64) for t in range(nsteps)]
    )
    j_arr = np.concatenate(
        [np.arange(nsteps - t, dtype=np.int64) for t in range(nsteps)]
    )
    gbs = NCORES * (nsteps - np.arange(nsteps, dtype=np.int64))
    goff = np.concatenate([[0], np.cumsum(gbs)])

    chunk_of_t = np.zeros(nsteps, np.int64)
    for ci in range(nchunks):
        chunk_of_t[CHUNK_TS[ci]:CHUNK_TS[ci + 1]] = ci
    ch_loc = chunk_of_t[t_arr]

    # per-core per-(chunk,rel) token counts -> exact shared tile budgets
    rel_by_core = []
    counts = np.zeros((NCORES, nchunks, R), np.int64)
    for core in range(NCORES):
        grow = goff[t_arr] + NCORES * j_arr + core
        rel_loc = rels[grow].astype(np.int64)
        rel_by_core.append((grow, rel_loc))
        np.add.at(counts[core], (ch_loc, rel_loc), 1)
    tiles_cr = [
        [int(np.ceil(counts[:, ci, r].max() / 128)) if counts[:, ci, r].max() > 0
         else 0 for r in range(R)]
        for ci in range(nchunks)
    ]

    # segment bases (must mirror build_program's tile order)
    seg_base = {}
    acc_tiles = 0
    for ci in range(nchunks):
        for r in range(R):
            seg_base[(ci, r)] = acc_tiles * 128
            acc_tiles += tiles_cr[ci][r]
    ntiles = acc_tiles

    in_maps = []
    for core in range(NCORES):
        grow, rel_loc = rel_by_core[core]
        node_loc = nodes[grow]

        order = np.lexsort((j_arr, t_arr, rel_loc, ch_loc))
        key = ch_loc[order] * R + rel_loc[order]
        cnt = np.bincount(key, minlength=nchunks * R)
        q = np.concatenate([np.arange(c) for c in cnt])
        base_sorted = np.array(
            [seg_base[(k // R, k % R)] for k in key], np.int64
        )
        prow_sorted = base_sorted + q
        prow = np.empty(nloc, np.int64)
        prow[order] = prow_sorted

        gidx_host = np.zeros((128, nsteps), np.int32)
        gidx_host[j_arr, t_arr] = prow

        Xp = np.zeros((ntiles * 128, D), np.float32)
        Xp[prow] = embed[node_loc]
        xt_host = np.ascontiguousarray(
            Xp.reshape(ntiles, 128, KD, 128).transpose(0, 3, 2, 1)
        ).astype(NPBF16)                         # [NT, 128(dk), KD, 128(tok)]

        in_maps.append({
            "xt": xt_host,
            "wf": wf_host,
            "wh": wh_host,
            "bt": bt_host,
            "brep": brep_host,
            "gidx": gidx_host,
        })

    unshard = {
        "t_arr": t_arr, "j_arr": j_arr, "goff": goff,
        "nloc": nloc,
    }
    return in_maps, unshard, tiles_cr


def kernel(**inputs):
    global LAST_RESULTS
    import os

    # Verify the packed layout matches the hardcoded structure.
    bs_exp, ti_exp, bi_exp = _expected_layout()
    ok = (
        np.array_equal(np.asarray(inputs["batch_sizes"]), bs_exp)
        and np.array_equal(np.asarray(inputs["time_idx"]), ti_exp)
        and np.array_equal(np.asarray(inputs["batch_idx"]), bi_exp)
        and np.asarray(inputs["embed"]).shape == (50000, D)
    )
    if not ok:
        return _numpy_reference(**{k: np.asarray(v) for k, v in inputs.items()})

    in_maps, unshard, tiles_cr = _prepare_host(inputs)

    nc = build_program(tiles_cr)
    trace = bool(os.environ.get("KERNEL_TRACE"))
    res = bass_utils.run_bass_kernel_spmd(
        nc, in_maps, core_ids=list(range(NCORES)), trace=trace,
    )
    LAST_RESULTS = res

    t_arr = unshard["t_arr"]
    j_arr = unshard["j_arr"]
    goff = unshard["goff"]
    out_full = np.zeros((len(np.asarray(inputs["time_idx"])), D), np.float32)
    for core in range(NCORES):
        grow = goff[t_arr] + NCORES * j_arr + core
        out_full[grow] = np.asarray(res.results[core]["out"], np.float32)
    return out_full
